# revision 1
# baseline (speedup 1.0000x reference)
"""Trainium2 Bass kernel for nn_NUFFTLayerMultiChannel3D_Param_57801669869710.

Factored-NUFFT formulation (no FFTs, everything is matmuls + elementwise):
  The spreading kernel K[n,x,y,z] is separable: gx[n,x]*gy[n,y]*gz[la,z], and
  its (shifted) 3D DFT is the separable product of 1D DFTs ghat.  With
  Ghat_n = fftshift(fftn(K_n)) precomputed on the host (input-independent):

    A: t[c,la,m2]      = sum_lo f[c,la,lo] * Gxy[la,lo,m2]     (Gxy = gxh⊗gyh)
    B: fftv[c,m2,kz]   = sum_la t[c,la,m2] * gzh[la,kz]
       filtered        = fftv * w,  w = deconv * total(params)  (elementwise)
    C: u[c,m2,la]      = sum_kz filtered[c,m2,kz] * conj(gzh[la,kz])
    D: energy[c,la,lo] = (1/N^3) Re sum_m2 u[c,m2,la]*conj(Gxy[la,lo,m2])

  Hermitian symmetry (real input field) keeps only 17 of 32 kz planes with
  paired filter weights wA + wB.

Sharding over 8 cores: la (npoints lat) is split 4-per-core for A/B/C/D; the
partial fftv fields (and per-channel partial filter fields, one channel per
core) are summed with a single fp16 AllReduce; each core then filters and
un-grids its own la slice.  Host gathers the 8 disjoint la slices.

Matmul operands are fp16 (fp32 runs LOW_HIGH double-pass on the PE);
power-of-2 scales folded into the host constants keep everything in fp16
range (the deconv filter reaches ~1e9).  PSUM accumulation is fp32.
"""

import functools

import numpy as np

N = 32
NLAT, NLON = 32, 64
C = 8
NCORES = 8
LAPC = NLAT // NCORES        # la values per core = 4
KZH = 17                     # packed half-space kz planes
M2 = N * N                   # 1024
L = 2.0 * np.pi
TAU = 12.0 * (L / (2.0 * np.pi * N)) ** 2
FFTV_LEN = 34 * 8192         # 278528
TOT_LEN = 34 * M2            # 34816
AR_LEN = FFTV_LEN + 2 * TOT_LEN

SB = 0.25                    # scale folded into BZ (fftv partials)
SW = 2.0 ** -14              # scale folded into wdA/wdB
SG = 2.0 ** 16               # scale folded into GD (undoes SB*SW)


# ----------------------------------------------------------------- host math
@functools.lru_cache(maxsize=1)
def _host_constants():
    lat = np.linspace(-np.pi / 2, np.pi / 2, NLAT)
    lon = np.linspace(0.0, 2.0 * np.pi, NLON)
    la, lo = np.meshgrid(lat, lon, indexing="ij")
    x = np.cos(la) * np.cos(lo)
    y = np.cos(la) * np.sin(lo)
    z = np.sin(lat)
    xg = np.linspace(-np.pi, np.pi, N + 1)[:-1]

    def g(d):
        return (np.exp(-d ** 2 / (4 * TAU))
                + np.exp(-(d - L) ** 2 / (4 * TAU))
                + np.exp(-(d + L) ** 2 / (4 * TAU)))

    gx = g(x[..., None] - xg)                   # (NLAT, NLON, N)
    gy = g(y[..., None] - xg)
    gz = g(z[:, None] - xg)                     # (NLAT, N)

    def sdft(a):
        return np.fft.fftshift(np.fft.fft(a, axis=-1), axes=-1)

    gxh = sdft(gx)
    gyh = sdft(gy)
    gzh = sdft(gz)                              # (NLAT, N) complex

    kg = (2.0 * np.pi / L) * np.linspace(-(N // 2), N // 2, N)
    kx, ky, kz = np.meshgrid(kg, kg, kg, indexing="ij")
    k2 = kx * kx + ky * ky + kz * kz
    kmag = np.sqrt(k2)
    deconv = (np.pi / TAU) ** 1.5 * np.exp(k2 * TAU)

    planes = np.array(list(range(16, 32)) + [0])      # 17 shifted kz planes
    sig = (32 - np.arange(N)) % N                     # shifted-index map for -m

    kmA3 = kmag[:, :, planes]                         # (32, 32, 17)
    decA3 = deconv[:, :, planes]
    kmB3 = kmag[sig][:, sig][:, :, sig][:, :, planes]
    decB3 = deconv[sig][:, sig][:, :, sig][:, :, planes]
    selfp = np.zeros(KZH)
    selfp[0] = 1.0                                    # packed 0  = freq 0
    selfp[16] = 1.0                                   # packed 16 = freq -16
    decB3 = decB3 * (1.0 - selfp)[None, None, :]

    def canon(f3):   # (32ix, 32iy, 17kz) -> flat[(2kz+ri)*1024 + ix*32+iy]
        a = f3.reshape(M2, KZH).T                      # (17, 1024)
        return np.repeat(a[:, None, :], 2, axis=1).reshape(-1)   # (34816,)

    kmA = canon(kmA3).astype(np.float32).reshape(128, 272)
    kmB = canon(kmB3).astype(np.float32).reshape(128, 272)
    wdA = (canon(decA3) * SW).astype(np.float16).reshape(34, 1024)
    wdB = (canon(decB3) * SW).astype(np.float16).reshape(34, 1024)

    gzH = gzh[:, planes]                              # (NLAT, 17) complex

    GA_all, BZ_all, CZ_all, GD_all = [], [], [], []
    for g_ in range(NCORES):
        sl = slice(4 * g_, 4 * g_ + 4)
        Gxy = (gxh[sl][:, :, :, None] * gyh[sl][:, :, None, :]).reshape(4, NLON, M2)
        GRe = Gxy.real.astype(np.float32)
        GIm = Gxy.imag.astype(np.float32)

        # GA2[(lo + 64*lap), pair*2048 + ri*1024 + m2] = RI(Gxy[2*pair+lap])
        GA = np.zeros((128, 4096), np.float16)
        for pair in range(2):
            for lap in range(2):
                la_ = 2 * pair + lap
                GA[64 * lap:64 * lap + 64, pair * 2048:pair * 2048 + 1024] = GRe[la_]
                GA[64 * lap:64 * lap + 64, pair * 2048 + 1024:pair * 2048 + 2048] = GIm[la_]
        GA_all.append(GA)

        gzc = gzH[sl]                                  # (4, 17)
        gzR = (gzc.real * SB).astype(np.float32)
        gzI = (gzc.imag * SB).astype(np.float32)

        bz8 = np.zeros((8, 34), np.float32)            # [2la+ri, 2kz+ri']
        bz8[0::2, 0::2] = gzR
        bz8[1::2, 0::2] = -gzI
        bz8[0::2, 1::2] = gzI
        bz8[1::2, 1::2] = gzR
        BZ = np.zeros((128, 34), np.float16)
        for q in range(4):
            BZ[32 * q:32 * q + 8] = bz8
        BZ_all.append(BZ)

        gzRu = gzc.real.astype(np.float32)
        gzIu = gzc.imag.astype(np.float32)
        cz34 = np.zeros((34, 8), np.float32)           # [2kz+ri, 2la+ri']
        cz34[0::2, 0::2] = gzRu.T
        cz34[1::2, 0::2] = gzIu.T
        cz34[0::2, 1::2] = -gzIu.T
        cz34[1::2, 1::2] = gzRu.T
        CZ = np.zeros((128, 8), np.float16)
        CZ[0:34] = cz34
        CZ[64:98] = cz34
        CZ_all.append(CZ)

        # GD[kp, la*1024 + (ri*8+ctm)*64 + lo] = RI(Gxy[la,lo,kp*8+ctm])*SG/N^3
        GD = np.zeros((128, 4, 2, 8, 64), np.float32)
        scale = SG / (N ** 3)
        GRe5 = (GRe * scale).reshape(4, 64, 128, 8)    # (la, lo, kp, ctm)
        GIm5 = (GIm * scale).reshape(4, 64, 128, 8)
        GD[:, :, 0, :, :] = GRe5.transpose(2, 0, 3, 1)
        GD[:, :, 1, :, :] = GIm5.transpose(2, 0, 3, 1)
        GD_all.append(GD.reshape(128, 4096).astype(np.float16))

    return dict(kmA=kmA, kmB=kmB, wdA=wdA, wdB=wdB,
                GA=GA_all, BZ=BZ_all, CZ=CZ_all, GD=GD_all)


# ------------------------------------------------------------- bass builder
@functools.lru_cache(maxsize=1)
def _build_module():
    import concourse.bass as bass
    import concourse.bacc as bacc
    import concourse.tile as tile
    import concourse.mybir as mybir

    dt32 = mybir.dt.float32
    dt16 = mybir.dt.float16
    AF = mybir.ActivationFunctionType
    MUL = mybir.AluOpType.mult
    ADD = mybir.AluOpType.add
    nc = bacc.Bacc("TRN2", target_bir_lowering=False, debug=False,
                   num_devices=NCORES)

    f_in = nc.dram_tensor("f_in", [128, 32], dt16, kind="ExternalInput").ap()
    prm = nc.dram_tensor("prm", [128, 4], dt32, kind="ExternalInput").ap()
    GA = nc.dram_tensor("GA", [128, 4096], dt16, kind="ExternalInput").ap()
    BZ = nc.dram_tensor("BZ", [128, 34], dt16, kind="ExternalInput").ap()
    CZ = nc.dram_tensor("CZ", [128, 8], dt16, kind="ExternalInput").ap()
    GD = nc.dram_tensor("GD", [128, 4096], dt16, kind="ExternalInput").ap()
    kmA = nc.dram_tensor("kmA", [128, 272], dt32, kind="ExternalInput").ap()
    kmB = nc.dram_tensor("kmB", [128, 272], dt32, kind="ExternalInput").ap()
    wdA = nc.dram_tensor("wdA", [34, 1024], dt16, kind="ExternalInput").ap()
    wdB = nc.dram_tensor("wdB", [34, 1024], dt16, kind="ExternalInput").ap()
    out_e = nc.dram_tensor("out_e", [128, 64], dt32, kind="ExternalOutput").ap()

    with tile.TileContext(nc) as tc:
        with (
            tc.tile_pool(name="sb", bufs=1) as sb,
            tc.tile_pool(name="ps", bufs=8, space="PSUM") as ps,
            tc.tile_pool(name="dr", bufs=1, space="DRAM") as dr,
        ):
            # ---------------- forward-critical loads (A path) first
            s_f = sb.tile([128, 32], dt16)
            nc.sync.dma_start(s_f[:, :], f_in[:, :])
            s_GA = sb.tile([128, 4096], dt16)
            for pair in range(2):
                nc.sync.dma_start(s_GA[:, pair * 2048:(pair + 1) * 2048],
                                  GA[:, pair * 2048:(pair + 1) * 2048])
            s_BZ = sb.tile([128, 34], dt16)
            nc.sync.dma_start(s_BZ[:, :], BZ[:, :])
            s_kmA = sb.tile([128, 272], dt32)
            nc.sync.dma_start(s_kmA[:, :], kmA[:, :])
            s_kmB = sb.tile([128, 272], dt32)
            nc.sync.dma_start(s_kmB[:, :], kmB[:, :])
            s_prm = sb.tile([128, 4], dt32)
            nc.sync.dma_start(s_prm[:, :], prm[:, :])

            # Tiny dummy collective issued first: pays the one-time CC entry
            # barrier (~30us) concurrently with the forward compute, so the
            # real RS/AG later start without it.
            d_pre_in = dr.tile([64], dt16)
            d_pre_out = dr.tile([512], dt16, addr_space="Shared")
            s_pre = sb.tile([1, 64], dt16)
            nc.vector.memset(s_pre[:, :], 0.0)
            nc.sync.dma_start(d_pre_in[:], s_pre[:, :])
            nc.gpsimd.collective_compute(
                "AllGather",
                mybir.AluOpType.bypass,
                replica_groups=[list(range(NCORES))],
                ins=[d_pre_in[:].opt()],
                outs=[d_pre_out[:].opt()],
            )

            d_ARin = dr.tile([AR_LEN], dt16)
            d_ARout = dr.tile([AR_LEN], dt16, addr_space="Shared")
            d_t = dr.tile([65536], dt16)
            d_u = dr.tile([65536], dt16)

            # ---------------- derived params ([128,1] each, fp32)
            p_amp = s_prm[:, 0:1]
            p_sh = s_prm[:, 1:2]
            p_be = s_prm[:, 2:3]
            p_hy = s_prm[:, 3:4]
            s_der = sb.tile([128, 8], dt32)
            d_asq = s_der[:, 0:1]
            d_shsq = s_der[:, 1:2]
            d_s2a = s_der[:, 2:3]
            d_mssh = s_der[:, 3:4]
            d_h400 = s_der[:, 4:5]
            d_bea2 = s_der[:, 5:6]
            d_bea = s_der[:, 6:7]
            nc.vector.tensor_mul(d_asq, p_amp, p_amp)
            nc.vector.tensor_mul(d_shsq, p_sh, p_sh)
            nc.vector.tensor_add(d_s2a, d_asq, d_shsq)
            nc.vector.tensor_scalar_mul(d_mssh, p_sh, -1.0)
            nc.vector.tensor_scalar_mul(d_h400, p_hy, 400.0)
            nc.vector.tensor_mul(d_bea, p_be, p_amp)
            nc.vector.tensor_scalar_mul(d_bea2, d_bea, 2.0)

            def bc(ap):
                return ap.broadcast_to((128, 272))

            # ---------------- total-field partials [128,272] fp32 -> fp16
            # contribution = 2*be*amp*usq*(usq+s2a) / (d1*d2*(usq+400hy)),
            # d1 = (u+sh)^2+amp^2, d2 = (u-sh)^2+amp^2  (no cancellation)
            tot16 = {}
            for F, km in (("A", s_kmA), ("B", s_kmB)):
                d1 = sb.tile([128, 272], dt32, name=f"d1{F}")
                d2 = sb.tile([128, 272], dt32, name=f"d2{F}")
                usq = sb.tile([128, 272], dt32, name=f"usq{F}")
                dd = sb.tile([128, 272], dt32, name=f"dd{F}")
                r = sb.tile([128, 272], dt32, name=f"r{F}")
                e1 = sb.tile([128, 272], dt32, name=f"e1{F}")
                den = sb.tile([128, 272], dt32, name=f"den{F}")
                rec = sb.tile([128, 272], dt32, name=f"rec{F}")
                num = sb.tile([128, 272], dt32, name=f"num{F}")
                tF = sb.tile([128, 272], dt32, name=f"tF{F}")
                nc.scalar.activation(d1, km[:, :], AF.Square, bias=p_sh)
                nc.vector.tensor_tensor(d1, d1, bc(d_asq), ADD)
                nc.scalar.activation(d2, km[:, :], AF.Square, bias=d_mssh)
                nc.vector.tensor_tensor(d2, d2, bc(d_asq), ADD)
                nc.scalar.activation(usq, km[:, :], AF.Square)
                nc.vector.tensor_mul(dd, d1, d2)
                nc.vector.tensor_tensor(r, usq, bc(d_s2a), ADD)
                nc.vector.tensor_tensor(e1, usq, bc(d_h400), ADD)
                nc.vector.tensor_mul(den, dd, e1)
                nc.vector.reciprocal(rec, den)
                nc.vector.tensor_mul(num, usq, r)
                nc.vector.tensor_tensor(num, num, bc(d_bea2), MUL)
                nc.vector.tensor_mul(tF, num, rec)
                tot16[F] = tF

            nc.gpsimd.dma_start(d_ARin[FFTV_LEN:FFTV_LEN + TOT_LEN],
                                tot16["A"][:, :])
            nc.gpsimd.dma_start(d_ARin[FFTV_LEN + TOT_LEN:AR_LEN],
                                tot16["B"][:, :])

            # ---------------- stage A (block-diag la-pairs, K=128, M=16)
            # s_t rows 32*pair + 8*lap + c, free (ri, m2); cast to fp16 on dump
            s_t = sb.tile([64, 2048], dt32)
            for pair in range(2):
                for jj in range(2):
                    psA = ps.tile([16, 1024], dt32, tag="ps2", bufs=2,
                                  name=f"psA{pair}_{jj}")
                    for j2 in range(2):
                        j = 2 * jj + j2
                        mm_lastA = nc.tensor.matmul(
                            psA[:, 512 * j2:512 * (j2 + 1)],
                            s_f[:, 16 * pair:16 * pair + 16],
                            s_GA[:, pair * 2048 + 512 * j:
                                 pair * 2048 + 512 * (j + 1)],
                            start=True, stop=True,
                        )
                    eng = nc.vector.tensor_copy if (pair + jj) % 2 == 0 \
                        else nc.scalar.copy
                    eng(s_t[32 * pair:32 * pair + 16,
                            1024 * jj:1024 * (jj + 1)], psA[:, :])

            for la_ in range(4):
                row = 32 * (la_ // 2) + 8 * (la_ % 2)
                nc.gpsimd.dma_start(d_t[la_ * 16384:(la_ + 1) * 16384],
                                    s_t[row:row + 8, :])

            # PE keep-warm helper: dummy matmuls into a scratch psum tile
            from concourse.tile import add_dep_helper
            ps_warm = ps.tile([16, 512], dt32, tag="ps2", bufs=2,
                              name="ps_warm")

            def warm(n, after=None):
                for i in range(n):
                    mm = nc.tensor.matmul(ps_warm[:, :], s_f[:, 0:16],
                                          s_GA[:, 0:512], start=True, stop=True)
                    if i == 0 and after is not None:
                        add_dep_helper(mm.ins, after.ins, sync=False,
                                       reason="keep-warm ordering")

            warm(6, after=mm_lastA)     # hold HAM through the A->B shuffle

            # rB[32q + 2la+ri, cq*1024 + m2], c = 2q+cq
            s_rB = sb.tile([128, 2048], dt16)
            v_t = d_t.rearrange("(la c ri m) -> ri la c m",
                                la=4, c=8, ri=2, m=1024)
            for q in range(4):
                for ri in range(2):
                    # partitions 32q+ri, step 2 over la
                    dst = s_rB[32 * q + ri:32 * q + ri + 7:2, :]
                    nc.sync.dma_start(dst, v_t[ri, :, 2 * q:2 * q + 2, :])

            # ---------------- stage B: fftv[2kz+ri', c*1024+m2] (row-tiled)
            # per (q): N = 2048 = (cq, m2); psum tiles of 1024 (= one c)
            s_fftv = sb.tile([34, 8192], dt32)
            for cq in range(2):
                for q in range(4):
                    psB = ps.tile([34, 1024], dt32, tag="ps2", bufs=2,
                                  name=f"psB{q}_{cq}")
                    for j2 in range(2):
                        mm_lastB = nc.tensor.matmul(
                            psB[:, 512 * j2:512 * (j2 + 1)],
                            s_BZ[32 * q:32 * q + 8, :],
                            s_rB[32 * q:32 * q + 8,
                                 1024 * cq + 512 * j2:1024 * cq + 512 * (j2 + 1)],
                            start=True, stop=True,
                            tile_position=(32 * q, 0),
                        )
                    col = (2 * q + cq) * 1024
                    eng = nc.vector.tensor_copy if (q + cq) % 2 == 0 \
                        else nc.scalar.copy
                    eng(s_fftv[:, col:col + 1024], psB[:, :])

            nc.gpsimd.dma_start(d_ARin[0:FFTV_LEN], s_fftv[:, :])

            # ---------------- AllReduce as ReduceScatter + AllGather
            # (RDH AllReduce measured ~19 GB/s algBW; RS+AG is much faster)
            SH = AR_LEN // NCORES
            d_rs = dr.tile([SH], dt16)
            nc.gpsimd.collective_compute(
                "ReduceScatter",
                mybir.AluOpType.add,
                replica_groups=[list(range(NCORES))],
                ins=[d_ARin[:].opt()],
                outs=[d_rs[:].opt()],
            )
            nc.gpsimd.collective_compute(
                "AllGather",
                mybir.AluOpType.bypass,
                replica_groups=[list(range(NCORES))],
                ins=[d_rs[:].opt()],
                outs=[d_ARout[:].opt()],
            )

            # backward-only constants (needed post-AR; loaded during AR)
            s_CZ = sb.tile([128, 8], dt16)
            nc.sync.dma_start(s_CZ[:, :], CZ[:, :])
            s_GD = sb.tile([128, 4096], dt16)
            nc.sync.dma_start(s_GD[:, :], GD[:, :])
            s_wdA = sb.tile([128, 1024], dt16)
            s_wdB = sb.tile([128, 1024], dt16)
            for rep in range(2):
                nc.sync.dma_start(s_wdA[64 * rep:64 * rep + 34, :], wdA[:, :])
                nc.sync.dma_start(s_wdB[64 * rep:64 * rep + 34, :], wdB[:, :])

            # ---------------- post-AR loads + filter
            v_fv = d_ARout[0:FFTV_LEN].rearrange("(kr cm) -> kr cm", kr=34)
            v_tA = d_ARout[FFTV_LEN:FFTV_LEN + TOT_LEN].rearrange(
                "(kr m) -> kr m", kr=34)
            v_tB = d_ARout[FFTV_LEN + TOT_LEN:AR_LEN].rearrange(
                "(kr m) -> kr m", kr=34)

            s_fil = sb.tile([128, 4096], dt16)
            s_fil2 = sb.tile([128, 4096], dt16)
            s_tsA = sb.tile([128, 1024], dt16)
            s_tsB = sb.tile([128, 1024], dt16)
            s_wt = sb.tile([128, 1024], dt16)
            for rep in range(2):
                sl = slice(64 * rep, 64 * rep + 34)
                nc.sync.dma_start(s_fil[sl, :],
                                  v_fv[:, 4096 * rep:4096 * (rep + 1)])
                nc.sync.dma_start(s_tsA[sl, :], v_tA[:, :])
                nc.sync.dma_start(s_tsB[sl, :], v_tB[:, :])
                nc.vector.tensor_mul(s_wt[sl, :], s_wdA[sl, :], s_tsA[sl, :])
                nc.vector.tensor_mul(s_tsB[sl, :], s_wdB[sl, :], s_tsB[sl, :])
                nc.vector.tensor_add(s_wt[sl, :], s_wt[sl, :], s_tsB[sl, :])
                for cc in range(4):       # contiguous per-c ops: DVE 2x mode
                    nc.vector.tensor_mul(
                        s_fil2[sl, 1024 * cc:1024 * (cc + 1)],
                        s_fil[sl, 1024 * cc:1024 * (cc + 1)],
                        s_wt[sl, :])

            # ---------------- stage C (col-tiled: 4 c per psum tile)
            # psC[rep][32*jc + (2la+ri'), m2], c = 4*rep + jc
            s_uE = sb.tile([128, 2048], dt32)      # [32jc+(2la+ri), rep*1024+m2]
            mm_lastC = None
            for rep in range(2):
                sl = slice(64 * rep, 64 * rep + 34)
                psC = ps.tile([128, 1024], dt32, tag="ps2", bufs=2,
                              name=f"psC{rep}")
                nc.vector.memset(psC[:, :], 0.0)
                for jc in range(4):
                    for j2 in range(2):
                        mm_lastC = nc.tensor.matmul(
                            psC[32 * jc:32 * jc + 8, 512 * j2:512 * (j2 + 1)],
                            s_CZ[sl, :],
                            s_fil2[sl, 1024 * jc + 512 * j2:
                                   1024 * jc + 512 * (j2 + 1)],
                            start=True, stop=True,
                            tile_position=(64 * rep, 32 * jc),
                        )
                eng = nc.vector.tensor_copy if rep == 0 else nc.scalar.copy
                eng(s_uE[:, 1024 * rep:1024 * (rep + 1)], psC[:, :])

            # dump: d_u[c*8192 + (2la+ri)*1024 + m2] (cast to fp16)
            for rep in range(2):
                for jc in range(4):
                    c_ = 4 * rep + jc
                    nc.gpsimd.dma_start(
                        d_u[c_ * 8192:(c_ + 1) * 8192],
                        s_uE[32 * jc:32 * jc + 8,
                             1024 * rep:1024 * (rep + 1)])

            warm(8, after=mm_lastC)   # bridge the C->D dump/load gap

            # reload for D: uD2[kp, la*128 + c*16 + ri*8 + ctm]
            s_uD = sb.tile([128, 512], dt16)
            v_uD = s_uD.rearrange("p (la c ri ctm) -> p la c ri ctm",
                                  la=4, c=8, ri=2, ctm=8)
            v_du = d_u.rearrange("(c la ri kp ctm) -> kp la c ri ctm",
                                 c=8, la=4, ri=2, kp=128, ctm=8)
            for la_ in range(4):
                for ri in range(2):
                    nc.sync.dma_start(v_uD[:, la_, :, ri, :],
                                      v_du[:, la_, :, ri, :])

            # ---------------- stage D (4 la col-tiled, 16-chunk accumulate)
            s_out = sb.tile([128, 64], dt32)
            psD = [ps.tile([128, 64], dt32, tag="psd", bufs=4,
                           name=f"psD{la_}")
                   for la_ in range(4)]
            for ct in range(16):
                for la_ in range(4):
                    nc.tensor.matmul(
                        psD[la_][32 * la_:32 * la_ + 8, :],
                        v_uD[:, la_, :, ct // 8, ct % 8],
                        s_GD[:, la_ * 1024 + ct * 64:la_ * 1024 + ct * 64 + 64],
                        start=(ct == 0), stop=(ct == 15),
                        tile_position=(0, 32 * la_),
                    )
            for la_ in range(4):
                eng_copy = (nc.vector.tensor_copy if la_ % 2 == 0
                            else nc.scalar.copy)
                eng_copy(s_out[32 * la_:32 * la_ + 8, :],
                         psD[la_][32 * la_:32 * la_ + 8, :])

            for la_ in range(4):
                nc.sync.dma_start(out_e[32 * la_:32 * la_ + 8, :],
                                  s_out[32 * la_:32 * la_ + 8, :])

    nc.compile()
    return nc


def _make_in_maps(inp, amplitude, shift, beta, hypera):
    consts = _host_constants()
    inp = np.ascontiguousarray(np.asarray(inp, np.float32))
    prms = [np.asarray(a, np.float32).reshape(-1) for a in
            (amplitude, shift, beta, hypera)]
    in_maps = []
    for g_ in range(NCORES):
        # block-diag f: [lo + 64*lap, pair*16 + 8*lap' + c], nonzero lap'==lap
        f = np.zeros((128, 32), np.float16)
        for pair in range(2):
            for lap in range(2):
                la_ = 2 * pair + lap
                f[64 * lap:64 * lap + 64, pair * 16 + 8 * lap:pair * 16 + 8 * lap + 8] = \
                    inp[0, :, 4 * g_ + la_, :].T
        prm = np.zeros((128, 4), np.float32)
        prm[:, 0] = prms[0][g_]
        prm[:, 1] = prms[1][g_]
        prm[:, 2] = prms[2][g_]
        prm[:, 3] = prms[3][g_]
        in_maps.append({
            "f_in": f,
            "prm": prm,
            "GA": consts["GA"][g_],
            "BZ": consts["BZ"][g_],
            "CZ": consts["CZ"][g_],
            "GD": consts["GD"][g_],
            "kmA": consts["kmA"],
            "kmB": consts["kmB"],
            "wdA": consts["wdA"],
            "wdB": consts["wdB"],
        })
    return in_maps


def _assemble(outs):
    energy = np.zeros((C, NLAT, NLON), np.float32)
    for g_ in range(NCORES):
        oe = outs[g_]["out_e"]                  # (128, 64)
        for la_ in range(LAPC):
            energy[:, 4 * g_ + la_, :] = oe[32 * la_:32 * la_ + 8, :]
    pred = energy.reshape(C, NLAT * NLON).T.reshape(1, C, NLAT, NLON)
    return pred


def kernel(inp, amplitude, shift, beta, hypera, _trace=False,
           _trace_cores=None):
    from concourse.bass_utils import run_bass_kernel_spmd

    nc = _build_module()
    in_maps = _make_in_maps(inp, amplitude, shift, beta, hypera)
    res = run_bass_kernel_spmd(nc, in_maps, core_ids=list(range(NCORES)),
                               trace=_trace, trace_cores=_trace_cores)
    out = _assemble(res.results)
    if _trace:
        kernel.last_results = res
    return out



# revision 11
# speedup vs baseline: 1.2361x; 1.2361x over previous
"""Trainium2 Bass kernel for nn_NUFFTLayerMultiChannel3D_Param_57801669869710.

Factored-NUFFT formulation (no FFTs, everything is matmuls + elementwise):
  The spreading kernel K[n,x,y,z] is separable: gx[n,x]*gy[n,y]*gz[la,z], and
  its (shifted) 3D DFT is the separable product of 1D DFTs.  With
  Ghat_n = fftshift(fftn(K_n)) precomputed on the host (input-independent):

    A: t[c,la,m2]      = sum_lo f[c,la,lo] * Gxy[la,lo,m2]     (Gxy = gxh⊗gyh)
    B: fftv[c,m2,kz]   = sum_la t[c,la,m2] * gzh[la,kz]
       filtered        = fftv * w,  w = deconv * total(params)  (elementwise)
    C: u[c,m2,la]      = sum_kz filtered[c,m2,kz] * conj(gzh[la,kz])
    D: energy[c,la,lo] = (1/N^3) Re sum_m2 u[c,m2,la]*conj(Gxy[la,lo,m2])

  Hermitian symmetry (real input field) keeps only 17 of 32 kz planes with
  paired filter weights wA + wB.

Sharding over 8 cores, ReduceScatter-only design:
  Forward: la (npoints lat) split 4-per-core for A/B; the partial fftv fields
  are summed AND m2-sharded in one fp16 ReduceScatter (AR buffer laid out
  [m2-shard, kz, c, m2in] so each core's RS segment is exactly its backward
  working set).  The filter weight w = deconv*total(params) is computed
  REDUNDANTLY on every core (params are tiny) during the RS wait, so it
  contributes no collective bytes and no post-collective dependency.
  Backward: each core filters + un-grids its own m2 shard for ALL 32 la
  (C contracts kz, D contracts the m2 shard), producing partial energies
  [32la, 8c, 64lo] that a small fp32 ReduceScatter sums and scatters so each
  core ends with its own 4-la slice = final output.  No AllGather, u never
  leaves SBUF.

Matmul operands are fp16 (fp32 runs LOW_HIGH double-pass on the PE);
power-of-2 scales folded into the host constants keep everything in fp16
range (the deconv filter reaches ~1e9).  PSUM accumulation is fp32.
"""

import functools

import numpy as np

N = 32
NLAT, NLON = 32, 64
C = 8
NCORES = 8
LAPC = NLAT // NCORES        # la values per core = 4
KZH = 17                     # packed half-space kz planes
M2 = N * N                   # 1024
SHARD = M2 // NCORES         # m2 values per core = 128
L = 2.0 * np.pi
TAU = 12.0 * (L / (2.0 * np.pi * N)) ** 2
FFTV_LEN = 34 * 8192         # 278528 = AR1 length
EIN_LEN = NLAT * C * NLON    # 16384 = AR2 length (fp32)

SB = 0.25                    # scale folded into BZ (fftv partials)
SW = 2.0 ** -14              # scale folded into wdA/wdB
SG = 2.0 ** 16               # scale folded into GDn (undoes SB*SW)


# ----------------------------------------------------------------- host math
@functools.lru_cache(maxsize=1)
def _host_constants():
    lat = np.linspace(-np.pi / 2, np.pi / 2, NLAT)
    lon = np.linspace(0.0, 2.0 * np.pi, NLON)
    la, lo = np.meshgrid(lat, lon, indexing="ij")
    x = np.cos(la) * np.cos(lo)
    y = np.cos(la) * np.sin(lo)
    z = np.sin(lat)
    xg = np.linspace(-np.pi, np.pi, N + 1)[:-1]

    def g(d):
        return (np.exp(-d ** 2 / (4 * TAU))
                + np.exp(-(d - L) ** 2 / (4 * TAU))
                + np.exp(-(d + L) ** 2 / (4 * TAU)))

    gx = g(x[..., None] - xg)                   # (NLAT, NLON, N)
    gy = g(y[..., None] - xg)
    gz = g(z[:, None] - xg)                     # (NLAT, N)

    def sdft(a):
        return np.fft.fftshift(np.fft.fft(a, axis=-1), axes=-1)

    gxh = sdft(gx)
    gyh = sdft(gy)
    gzh = sdft(gz)                              # (NLAT, N) complex

    kg = (2.0 * np.pi / L) * np.linspace(-(N // 2), N // 2, N)
    kx, ky, kz = np.meshgrid(kg, kg, kg, indexing="ij")
    k2 = kx * kx + ky * ky + kz * kz
    kmag = np.sqrt(k2)
    deconv = (np.pi / TAU) ** 1.5 * np.exp(k2 * TAU)

    planes = np.array(list(range(16, 32)) + [0])      # 17 shifted kz planes
    sig = (32 - np.arange(N)) % N                     # shifted-index map for -m

    kmA3 = kmag[:, :, planes]                         # (32, 32, 17)
    decA3 = deconv[:, :, planes]
    kmB3 = kmag[sig][:, sig][:, :, sig][:, :, planes]
    decB3 = deconv[sig][:, sig][:, :, sig][:, :, planes]
    selfp = np.zeros(KZH)
    selfp[0] = 1.0                                    # packed 0  = freq 0
    selfp[16] = 1.0                                   # packed 16 = freq -16
    decB3 = decB3 * (1.0 - selfp)[None, None, :]

    def half(f3):    # (32ix, 32iy, 17kz) -> [17, 1024] (m2 = ix*32+iy)
        return f3.reshape(M2, KZH).T.copy()

    usqA = half(kmA3) ** 2                            # (17, 1024)
    usqB = half(kmB3) ** 2
    wdA2 = half(decA3) * SW                           # (17, 1024)
    wdB2 = half(decB3) * SW

    gzH = gzh[:, planes]                              # (NLAT, 17) complex

    # CZF [34, 64]: stage-C weights for ALL 32 la.
    gzR = gzH.real.astype(np.float32)
    gzI = gzH.imag.astype(np.float32)
    CZF = np.zeros((34, 64), np.float32)
    CZF[0::2, 0::2] = gzR.T
    CZF[1::2, 0::2] = gzI.T
    CZF[0::2, 1::2] = -gzI.T
    CZF[1::2, 1::2] = gzR.T
    CZF = CZF.astype(np.float16)

    IDT = np.eye(128, dtype=np.float16)

    # full Gxy needed for the m2-sharded GDn tensors
    Gxy_full = gxh[:, :, :, None] * gyh[:, :, None, :]   # (la, lo, kx, ky)
    Gxy_full = Gxy_full.reshape(NLAT, NLON, M2)

    GA_all, BZ_all, GDn_all, usqC_all = [], [], [], []
    wdAp_all, wdBp_all = [], []
    for g_ in range(NCORES):
        sl = slice(4 * g_, 4 * g_ + 4)
        Gxy = Gxy_full[sl]                             # (4, NLON, M2)
        GRe = Gxy.real.astype(np.float32)
        GIm = Gxy.imag.astype(np.float32)

        # GA[(lo + 64*lap), pair*2048 + ri*1024 + m2] = RI(Gxy[2*pair+lap])
        GA = np.zeros((128, 4096), np.float16)
        for pair in range(2):
            for lap in range(2):
                la_ = 2 * pair + lap
                GA[64 * lap:64 * lap + 64, pair * 2048:pair * 2048 + 1024] = GRe[la_]
                GA[64 * lap:64 * lap + 64, pair * 2048 + 1024:pair * 2048 + 2048] = GIm[la_]
        GA_all.append(GA)

        gzc = gzH[sl]                                  # (4, 17)
        gzRs = (gzc.real * SB).astype(np.float32)
        gzIs = (gzc.imag * SB).astype(np.float32)
        bz8 = np.zeros((8, 34), np.float32)            # [2la+ri, 2kz+ri']
        bz8[0::2, 0::2] = gzRs
        bz8[1::2, 0::2] = -gzIs
        bz8[0::2, 1::2] = gzIs
        bz8[1::2, 1::2] = gzRs
        BZ = np.zeros((128, 34), np.float16)
        for q in range(4):
            BZ[32 * q:32 * q + 8] = bz8
        BZ_all.append(BZ)

        # GDn[m2in, (2la+ri)*64 + lo] = RI(Gxy[la, lo, 128g+m2in]) * SG/N^3
        msl = slice(SHARD * g_, SHARD * (g_ + 1))
        scale = SG / (N ** 3)
        GshR = (Gxy_full.real[:, :, msl] * scale).astype(np.float32)  # (32,64,128)
        GshI = (Gxy_full.imag[:, :, msl] * scale).astype(np.float32)
        GDn = np.zeros((SHARD, NLAT, 2, NLON), np.float32)
        GDn[:, :, 0, :] = GshR.transpose(2, 0, 1)
        GDn[:, :, 1, :] = GshI.transpose(2, 0, 1)
        GDn_all.append(GDn.reshape(SHARD, 4096).astype(np.float16))

        # usqC [128, 272]: col X*136 + c*17 + k, rows = m2in (shard)
        usqC = np.zeros((SHARD, 272), np.float32)
        uA = usqA[:, msl].T                            # (128, 17)
        uB = usqB[:, msl].T
        for c in range(C):
            usqC[:, c * 17:(c + 1) * 17] = uA
            usqC[:, 136 + c * 17:136 + (c + 1) * 17] = uB
        usqC_all.append(usqC)

        wdAp_all.append(wdA2[:, msl].T.astype(np.float16))   # (128, 17)
        wdBp_all.append(wdB2[:, msl].T.astype(np.float16))

    return dict(GA=GA_all, BZ=BZ_all, GDn=GDn_all, usqC=usqC_all,
                wdAp=wdAp_all, wdBp=wdBp_all, CZF=CZF, IDT=IDT)


# ------------------------------------------------------------- bass builder
@functools.lru_cache(maxsize=1)
def _build_module():
    import concourse.bass as bass
    import concourse.bacc as bacc
    import concourse.tile as tile
    import concourse.mybir as mybir

    dt32 = mybir.dt.float32
    dt16 = mybir.dt.float16
    MUL = mybir.AluOpType.mult
    ADD = mybir.AluOpType.add
    SUB = mybir.AluOpType.subtract
    nc = bacc.Bacc("TRN2", target_bir_lowering=False, debug=False,
                   num_devices=NCORES)

    f_in = nc.dram_tensor("f_in", [128, 32], dt16, kind="ExternalInput").ap()
    GA = nc.dram_tensor("GA", [128, 4096], dt16, kind="ExternalInput").ap()
    BZ = nc.dram_tensor("BZ", [128, 34], dt16, kind="ExternalInput").ap()
    CZF = nc.dram_tensor("CZF", [34, 64], dt16, kind="ExternalInput").ap()
    GDn = nc.dram_tensor("GDn", [128, 4096], dt16, kind="ExternalInput").ap()
    usqC = nc.dram_tensor("usqC", [128, 272], dt32, kind="ExternalInput").ap()
    ampT = nc.dram_tensor("ampT", [128, 272], dt32, kind="ExternalInput").ap()
    shT = nc.dram_tensor("shT", [128, 272], dt32, kind="ExternalInput").ap()
    beT = nc.dram_tensor("beT", [128, 272], dt32, kind="ExternalInput").ap()
    hyT = nc.dram_tensor("hyT", [128, 272], dt32, kind="ExternalInput").ap()
    wdAp = nc.dram_tensor("wdAp", [128, 17], dt16, kind="ExternalInput").ap()
    wdBp = nc.dram_tensor("wdBp", [128, 17], dt16, kind="ExternalInput").ap()
    IDT = nc.dram_tensor("IDT", [128, 128], dt16, kind="ExternalInput").ap()
    pre_in = nc.dram_tensor("pre_in", [1, 64], dt16, kind="ExternalInput").ap()
    out_e = nc.dram_tensor("out_e", [32, 64], dt32, kind="ExternalOutput").ap()

    with tile.TileContext(nc) as tc:
        with (
            tc.tile_pool(name="sb", bufs=1) as sb,
            tc.tile_pool(name="ps", bufs=8, space="PSUM") as ps,
            tc.tile_pool(name="dr", bufs=1, space="DRAM") as dr,
        ):
            # ---------------- forward-critical loads (A path) first
            s_f = sb.tile([128, 32], dt16)
            nc.sync.dma_start(s_f[:, :], f_in[:, :])
            s_GA = sb.tile([128, 4096], dt16)
            for pair in range(2):
                nc.sync.dma_start(s_GA[:, pair * 2048:(pair + 1) * 2048],
                                  GA[:, pair * 2048:(pair + 1) * 2048])
            s_BZ = sb.tile([128, 34], dt16)
            nc.sync.dma_start(s_BZ[:, :], BZ[:, :])

            # Tiny dummy collective issued first: pays the one-time CC entry
            # barrier concurrently with the forward compute.  (Collectives
            # cannot read IO tensors, so bounce the constant through SBUF.)
            s_pre = sb.tile([1, 64], dt16)
            nc.sync.dma_start(s_pre[:, :], pre_in[:, :])
            d_pre_in = dr.tile([64], dt16)
            nc.gpsimd.dma_start(d_pre_in[:], s_pre[:, :])
            d_pre_out = dr.tile([512], dt16, addr_space="Shared")
            nc.gpsimd.collective_compute(
                "AllGather",
                mybir.AluOpType.bypass,
                replica_groups=[list(range(NCORES))],
                ins=[d_pre_in[:].opt()],
                outs=[d_pre_out[:].opt()],
            )

            d_ARin = dr.tile([FFTV_LEN], dt16)
            d_rs = dr.tile([FFTV_LEN // NCORES], dt16)
            d_Ein = dr.tile([EIN_LEN], dt32)
            d_rs2 = dr.tile([EIN_LEN // NCORES], dt32)

            # backward constants on the scalar queue (off the sync queue so
            # they don't contend with the forward-critical path)
            s_CZF = sb.tile([34, 64], dt16)
            nc.scalar.dma_start(s_CZF[:, :], CZF[:, :])
            s_IDT = sb.tile([128, 128], dt16)
            nc.scalar.dma_start(s_IDT[:, :], IDT[:, :])
            s_usq = sb.tile([128, 272], dt32)
            nc.scalar.dma_start(s_usq[:, :], usqC[:, :])
            s_ampT = sb.tile([128, 272], dt32)
            nc.scalar.dma_start(s_ampT[:, :], ampT[:, :])
            s_shT = sb.tile([128, 272], dt32)
            nc.scalar.dma_start(s_shT[:, :], shT[:, :])
            s_beT = sb.tile([128, 272], dt32)
            nc.scalar.dma_start(s_beT[:, :], beT[:, :])
            s_hyT = sb.tile([128, 272], dt32)
            nc.scalar.dma_start(s_hyT[:, :], hyT[:, :])
            s_wdAp = sb.tile([128, 17], dt16)
            nc.scalar.dma_start(s_wdAp[:, :], wdAp[:, :])
            s_wdBp = sb.tile([128, 17], dt16)
            nc.scalar.dma_start(s_wdBp[:, :], wdBp[:, :])
            s_GDn = sb.tile([128, 4096], dt16)
            nc.scalar.dma_start(s_GDn[:, :], GDn[:, :])

            # ---------------- stage A (block-diag la-pairs, K=128, M=16)
            # s_t rows 32*pair + 8*lap + c, free (ri, m2); fp16 so the
            # shuffle DMAs below are cast-free (casts are gpsimd-only)
            s_t = sb.tile([64, 2048], dt16)
            for pair in range(2):
                for jj in range(2):
                    psA = ps.tile([16, 1024], dt32, tag="ps2", bufs=2,
                                  name=f"psA{pair}_{jj}")
                    for j2 in range(2):
                        j = 2 * jj + j2
                        mm_lastA = nc.tensor.matmul(
                            psA[:, 512 * j2:512 * (j2 + 1)],
                            s_f[:, 16 * pair:16 * pair + 16],
                            s_GA[:, pair * 2048 + 512 * j:
                                 pair * 2048 + 512 * (j + 1)],
                            start=True, stop=True,
                        )
                    eng = nc.vector.tensor_copy if (pair + jj) % 2 == 0 \
                        else nc.scalar.copy
                    eng(s_t[32 * pair:32 * pair + 16,
                            1024 * jj:1024 * (jj + 1)], psA[:, :])

            # PE keep-warm helper: dummy matmuls into a scratch psum tile
            from concourse.tile import add_dep_helper
            ps_warm = ps.tile([16, 512], dt32, tag="ps2", bufs=2,
                              name="ps_warm")

            def warm(n, after=None):
                first = None
                for i in range(n):
                    mm = nc.tensor.matmul(ps_warm[:, :], s_f[:, 0:16],
                                          s_GA[:, 0:512], start=True, stop=True)
                    if i == 0:
                        first = mm
                        if after is not None:
                            add_dep_helper(mm.ins, after.ins, sync=False,
                                           reason="keep-warm ordering")
                return first

            warm(6, after=mm_lastA)     # hold HAM through the A->B shuffle

            # ---------------- A -> B shuffle through DRAM (fp16 both ways)
            # d_t layout [la, c, ri, m2]; rB[32q + 2la+ri, cq*1024 + m2]
            d_t = dr.tile([65536], dt16)
            for la_ in range(4):
                row = 32 * (la_ // 2) + 8 * (la_ % 2)
                nc.sync.dma_start(d_t[la_ * 16384:(la_ + 1) * 16384],
                                  s_t[row:row + 8, :])
            s_rB = sb.tile([128, 2048], dt16)
            v_t = d_t[:].rearrange("(la c ri m) -> ri la c m",
                                   la=4, c=8, ri=2, m=1024)
            for q in range(4):
                for ri in range(2):
                    nc.sync.dma_start(s_rB[32 * q + ri:32 * q + ri + 7:2, :],
                                      v_t[ri, :, 2 * q:2 * q + 2, :])

            # ---------------- stage B: fftv[2kz+ri', free (s, c, m2in)]
            s_fftv = sb.tile([34, 8192], dt16)
            v_ff = s_fftv[:, :].rearrange("p (s c m) -> p s c m",
                                          s=8, c=8, m=SHARD)
            for cq in range(2):
                for q in range(4):
                    psB = ps.tile([34, 1024], dt32, tag="ps2", bufs=2,
                                  name=f"psB{q}_{cq}")
                    for j2 in range(2):
                        mm_lastB = nc.tensor.matmul(
                            psB[:, 512 * j2:512 * (j2 + 1)],
                            s_BZ[32 * q:32 * q + 8, :],
                            s_rB[32 * q:32 * q + 8,
                                 1024 * cq + 512 * j2:1024 * cq + 512 * (j2 + 1)],
                            start=True, stop=True,
                            tile_position=(32 * q, 0),
                        )
                    c_ = 2 * q + cq
                    eng = nc.vector.tensor_copy if (q + cq) % 2 == 0 \
                        else nc.scalar.copy
                    eng(v_ff[:, :, c_, :],
                        psB[:, :].rearrange("p (s m) -> p s m", s=8))

            # dump: AR segment s = [kr, c, m2in] contiguous 34816
            v_AR = d_ARin[:].rearrange("(s kr cm) -> s kr cm", s=8, kr=34)
            for s_ in range(8):
                nc.gpsimd.dma_start(v_AR[s_, :, :],
                                    s_fftv[:, 1024 * s_:1024 * (s_ + 1)])

            # ---------------- single big ReduceScatter (fp16)
            nc.gpsimd.collective_compute(
                "ReduceScatter",
                mybir.AluOpType.add,
                replica_groups=[list(range(NCORES))],
                ins=[d_ARin[:].opt()],
                outs=[d_rs[:].opt()],
            )

            # ---------------- local filter weight w (during the RS wait)
            # layout [m2in=128 part, col = X*136 + c*17 + k], X = A/B variant
            s_asq = sb.tile([128, 272], dt32)
            s_shsq = sb.tile([128, 272], dt32)
            s_s2a = sb.tile([128, 272], dt32)
            s_m4s = sb.tile([128, 272], dt32)
            s_h400 = sb.tile([128, 272], dt32)
            s_bea2 = sb.tile([128, 272], dt32)
            nc.vector.tensor_mul(s_asq[:, :], s_ampT[:, :], s_ampT[:, :])
            nc.vector.tensor_mul(s_shsq[:, :], s_shT[:, :], s_shT[:, :])
            nc.vector.tensor_add(s_s2a[:, :], s_asq[:, :], s_shsq[:, :])
            nc.vector.tensor_scalar_mul(s_m4s[:, :], s_shsq[:, :], 4.0)
            nc.vector.tensor_scalar_mul(s_h400[:, :], s_hyT[:, :], 400.0)
            nc.vector.tensor_mul(s_bea2[:, :], s_beT[:, :], s_ampT[:, :])
            nc.vector.tensor_scalar_mul(s_bea2[:, :], s_bea2[:, :], 2.0)

            # contribution = 2*be*amp*usq*(usq+s2a) / (dd*(usq+400hy)),
            # dd = (usq+s2a)^2 - 4*sh^2*usq  (= d1*d2, no cancellation)
            s_r = sb.tile([128, 272], dt32)
            s_t1 = sb.tile([128, 272], dt32)
            s_t2 = sb.tile([128, 272], dt32)
            s_e1 = sb.tile([128, 272], dt32)
            s_den = sb.tile([128, 272], dt32)
            s_rec = sb.tile([128, 272], dt32)
            s_num = sb.tile([128, 272], dt32)
            s_tF = sb.tile([128, 272], dt32)
            nc.vector.tensor_add(s_r[:, :], s_usq[:, :], s_s2a[:, :])
            nc.vector.tensor_mul(s_t1[:, :], s_r[:, :], s_r[:, :])
            nc.vector.tensor_mul(s_t2[:, :], s_usq[:, :], s_m4s[:, :])
            nc.vector.tensor_tensor(s_t1[:, :], s_t1[:, :], s_t2[:, :], SUB)
            nc.vector.tensor_add(s_e1[:, :], s_usq[:, :], s_h400[:, :])
            nc.vector.tensor_mul(s_den[:, :], s_t1[:, :], s_e1[:, :])
            nc.vector.reciprocal(s_rec[:, :], s_den[:, :])
            nc.vector.tensor_mul(s_num[:, :], s_usq[:, :], s_r[:, :])
            nc.vector.tensor_mul(s_num[:, :], s_num[:, :], s_bea2[:, :])
            nc.vector.tensor_mul(s_tF[:, :], s_num[:, :], s_rec[:, :])

            # channel-sum: 3 pairwise adds per X variant -> [128, 17] each
            s_red = sb.tile([128, 136], dt32)
            s_wA = sb.tile([128, 17], dt32)
            s_wB = sb.tile([128, 17], dt32)
            for X, wdst in ((0, s_wA), (1, s_wB)):
                b0 = 136 * X
                nc.vector.tensor_add(s_red[:, 0:68],
                                     s_tF[:, b0:b0 + 68],
                                     s_tF[:, b0 + 68:b0 + 136])
                nc.vector.tensor_add(s_red[:, 68:102],
                                     s_red[:, 0:34], s_red[:, 34:68])
                nc.vector.tensor_add(wdst[:, :],
                                     s_red[:, 68:85], s_red[:, 85:102])

            # w = wdA*wA + wdB*wB  (fp16 consts cast up first)
            s_wd32 = sb.tile([128, 34], dt32)
            nc.vector.tensor_copy(s_wd32[:, 0:17], s_wdAp[:, :])
            nc.vector.tensor_copy(s_wd32[:, 17:34], s_wdBp[:, :])
            s_wsum = sb.tile([128, 17], dt32)
            nc.vector.tensor_mul(s_wA[:, :], s_wA[:, :], s_wd32[:, 0:17])
            nc.vector.tensor_mul(s_wB[:, :], s_wB[:, :], s_wd32[:, 17:34])
            nc.vector.tensor_add(s_wsum[:, :], s_wA[:, :], s_wB[:, :])
            s_w16 = sb.tile([128, 17], dt16)
            nc.vector.tensor_copy(s_w16[:, :], s_wsum[:, :])

            # transpose w to [17 kz, 128 m2in] on the PE, then duplicate the
            # ri pairs with two partition-strided SBUF->SBUF DMAs
            ps_w = ps.tile([17, 128], dt16, tag="psw", bufs=1, name="ps_w")
            nc.tensor.transpose(ps_w[:, :], s_w16[:, :], s_IDT[:, :])
            s_w17 = sb.tile([17, 128], dt16)
            nc.scalar.copy(s_w17[:, :], ps_w[:, :])
            s_w34 = sb.tile([34, 128], dt16)
            nc.scalar.dma_start(s_w34[0:33:2, :], s_w17[:, :])
            nc.scalar.dma_start(s_w34[1:34:2, :], s_w17[:, :])

            # ---------------- post-RS: load shard, filter, stage C
            s_fv = sb.tile([34, 1024], dt16)
            dma_fv = nc.sync.dma_start(
                s_fv[:, :], d_rs[:].rearrange("(kr cm) -> kr cm", kr=34))

            warm(4, after=dma_fv)     # ramp the PE while the filter runs

            s_fil = sb.tile([34, 1024], dt16)
            for c_ in range(C):
                nc.vector.tensor_mul(s_fil[:, 128 * c_:128 * (c_ + 1)],
                                     s_fv[:, 128 * c_:128 * (c_ + 1)],
                                     s_w34[:, :])

            # stage C: u[m2in=128, c*64 + (2la+ri)] = fil_c^T @ CZF
            ps_u = ps.tile([128, 512], dt32, tag="psu", bufs=1, name="ps_u")
            for c_ in range(C):
                nc.tensor.matmul(
                    ps_u[:, 64 * c_:64 * (c_ + 1)],
                    s_fil[:, 128 * c_:128 * (c_ + 1)],
                    s_CZF[:, :],
                    start=True, stop=True,
                )
            s_u = sb.tile([128, 512], dt16)
            nc.vector.tensor_copy(s_u[:, :], ps_u[:, :])

            # ---------------- stage D: partial energies for ALL 32 la
            # la = 16t + 4f + p: psD[t][32p + c, 128f + 64ri + lo], every
            # matmul single-shot (interleaved start/stop accumulation across
            # regions of one PE strip corrupts psum).  Matmul outputs must
            # start at 32-partition boundaries, hence the (p, f) split; the
            # ri pairs are summed on the DVE below.
            psD = [ps.tile([128, 512], dt32, tag="psd", bufs=2,
                           name=f"psD{t}") for t in range(2)]
            for la_ in range(NLAT):
                t, rem = divmod(la_, 16)
                f_, p_ = divmod(rem, 4)
                for ri in range(2):
                    j = 2 * la_ + ri
                    nc.tensor.matmul(
                        psD[t][32 * p_:32 * p_ + 8,
                               128 * f_ + 64 * ri:128 * f_ + 64 * (ri + 1)],
                        s_u[:, j:j + 449:64],
                        s_GDn[:, 64 * j:64 * (j + 1)],
                        start=True, stop=True,
                        tile_position=(0, 32 * p_),
                    )
            # s_ep[32p + c, 256t + 64f + lo] = psD[t][.., ri=0] + [.., ri=1]
            # (DVE may read only one PSUM operand: copy ri=0, then add ri=1)
            s_ep = sb.tile([128, 512], dt32)
            for t in range(2):
                v_ps = psD[t][:, :].rearrange("p (f ri lo) -> p f ri lo",
                                              f=4, ri=2)
                dst = s_ep[:, 256 * t:256 * (t + 1)] \
                    .rearrange("p (f lo) -> p f lo", f=4)
                eng = nc.scalar.copy if t == 0 else nc.vector.tensor_copy
                eng(dst, v_ps[:, :, 0, :])
                nc.vector.tensor_add(dst, dst, v_ps[:, :, 1, :])

            # d_Ein[g*2048 + p*512 + c*64 + lo]  (= la*512 + c*64 + lo)
            v_Ein = d_Ein[:].rearrange("(g p c lo) -> g p c lo",
                                       g=8, p=4, c=8)
            for g_ in range(8):
                t, f_ = divmod(g_, 4)
                for p_ in range(4):
                    nc.gpsimd.dma_start(
                        v_Ein[g_, p_, :, :],
                        s_ep[32 * p_:32 * p_ + 8,
                             256 * t + 64 * f_:256 * t + 64 * (f_ + 1)])

            # ---------------- small fp32 ReduceScatter of energy partials
            nc.gpsimd.collective_compute(
                "ReduceScatter",
                mybir.AluOpType.add,
                replica_groups=[list(range(NCORES))],
                ins=[d_Ein[:].opt()],
                outs=[d_rs2[:].opt()],
            )

            # out_e[8*la_loc + c, lo] <- d_rs2 (bounce via SBUF)
            s_out = sb.tile([32, 64], dt32)
            nc.sync.dma_start(
                s_out[:, :], d_rs2[:].rearrange("(r lo) -> r lo", r=32))
            nc.sync.dma_start(out_e[:, :], s_out[:, :])

    nc.compile()
    return nc


def _make_in_maps(inp, amplitude, shift, beta, hypera):
    consts = _host_constants()
    inp = np.ascontiguousarray(np.asarray(inp, np.float32))
    prms = [np.asarray(a, np.float32).reshape(-1) for a in
            (amplitude, shift, beta, hypera)]

    # params replicated into the [128, 272] (X, c, k) compute layout
    def ptile(v):
        row = np.tile(np.repeat(v, KZH), 2).astype(np.float32)   # (272,)
        return np.broadcast_to(row, (128, 272)).copy()

    ampT = ptile(prms[0])
    shT = ptile(prms[1])
    beT = ptile(prms[2])
    hyT = ptile(prms[3])
    pre = np.zeros((1, 64), np.float16)

    in_maps = []
    for g_ in range(NCORES):
        # block-diag f: [lo + 64*lap, pair*16 + 8*lap' + c], nonzero lap'==lap
        f = np.zeros((128, 32), np.float16)
        for pair in range(2):
            for lap in range(2):
                la_ = 2 * pair + lap
                f[64 * lap:64 * lap + 64, pair * 16 + 8 * lap:pair * 16 + 8 * lap + 8] = \
                    inp[0, :, 4 * g_ + la_, :].T
        in_maps.append({
            "f_in": f,
            "GA": consts["GA"][g_],
            "BZ": consts["BZ"][g_],
            "CZF": consts["CZF"],
            "GDn": consts["GDn"][g_],
            "usqC": consts["usqC"][g_],
            "ampT": ampT,
            "shT": shT,
            "beT": beT,
            "hyT": hyT,
            "wdAp": consts["wdAp"][g_],
            "wdBp": consts["wdBp"][g_],
            "IDT": consts["IDT"],
            "pre_in": pre,
        })
    return in_maps


def _assemble(outs):
    energy = np.zeros((C, NLAT, NLON), np.float32)
    for g_ in range(NCORES):
        oe = outs[g_]["out_e"]                  # (32, 64) = [la_loc*8 + c, lo]
        for j in range(LAPC):
            energy[:, 4 * g_ + j, :] = oe[8 * j:8 * j + 8, :]
    pred = energy.reshape(C, NLAT * NLON).T.reshape(1, C, NLAT, NLON)
    return pred


def kernel(inp, amplitude, shift, beta, hypera, _trace=False,
           _trace_cores=None):
    from concourse.bass_utils import run_bass_kernel_spmd

    nc = _build_module()
    in_maps = _make_in_maps(inp, amplitude, shift, beta, hypera)
    res = run_bass_kernel_spmd(nc, in_maps, core_ids=list(range(NCORES)),
                               trace=_trace, trace_cores=_trace_cores)
    out = _assemble(res.results)
    if _trace:
        kernel.last_results = res
    return out
